# revision 7
# baseline (speedup 1.0000x reference)
"""ContrastiveLoss distributed Trainium2 kernel (8 NeuronCores).

Math (reference):
  t = l2norm(textual); c0 = l2norm(f0) @ t.T; c1 = l2norm(f1) @ t.T
  loss = sum(lab*(1-c) + (1-lab)*relu(c-1)) over both c / B^2

Sharding: rows of f0/f1/labels split across 8 cores (512 rows each);
textual replicated. Per core, per similarity matrix:
  sum = sum(lab) - sum(lab*c) + sum(r) - sum(lab*r),   r = relu(c-1)
Per-core partial sums returned as [128,1] per-partition totals; host sums.

Device layout: host ships bf16 transposed operands (layout/dtype marshalling
only; all float math on device):
  x{0,1}n [512,1024]  natural (row sumsq via ACT Square+accum)
  x{0,1}t [1024,512]  transposed (matmul lhsT tiles, unnormalized)
  tt      [1024,4096] transposed textual (matmul rhs, normalized in-place)
  lab     [512,4096]  labels slice
X normalization is folded into the per-partition scale operand of the
ACT/DVE passes over PSUM (c_raw*fx = c), so x*t tiles stay unnormalized.
"""
import sys

if "/opt/trn_rl_repo" not in sys.path:
    sys.path.insert(0, "/opt/trn_rl_repo")

import numpy as np
import ml_dtypes

import concourse.bass as bass
import concourse.mybir as mybir
import concourse.tile as tile
from concourse import bass_utils
import bass_rust

B, D = 4096, 1024
NCORES = 8
RPC = B // NCORES          # rows per core = 512
P = 128
ITILES = RPC // P          # 4
KT = D // P                # 8 contraction tiles
NJ = B // 512              # 8 j-chunks of 512
bf16 = mybir.dt.bfloat16
f32 = mybir.dt.float32
EPS = 1e-8

_CACHE = {}


def _split_waits(nc, max_waits=1):
    """This walrus build rejects >1 semaphore wait per instruction; hoist
    extras onto same-engine NOPs placed immediately before."""
    SI = bass_rust.SyncInfo
    n = 0
    for bb in nc.main_func.blocks:
        new_insts, changed = [], False
        for inst in bb.instructions:
            si = inst.sync_info
            if si is None:
                new_insts.append(inst)
                continue
            waits = list(si.on_wait)
            if len(waits) > max_waits:
                extra, keep = waits[:-max_waits], waits[-max_waits:]
                for j in range(0, len(extra), max_waits):
                    nop = mybir.InstNoOp(name=f"{inst.name}-ws{j}", ins=[], outs=[])
                    nop.engine = inst.engine
                    nop.sync_info = SI(on_wait=extra[j : j + max_waits], on_update=[])
                    nc.register_instruction(nop, overwrite=True)
                    new_insts.append(nop)
                    n += 1
                inst.sync_info = SI(on_wait=keep, on_update=list(si.on_update))
                changed = True
            new_insts.append(inst)
        if changed:
            bb.instructions = new_insts
    return n


def _build():
    nc = bass.Bass("TRN2", target_bir_lowering=False, debug=False,
                   num_devices=NCORES)
    A = mybir.AluOpType
    AF = mybir.ActivationFunctionType

    x0n = nc.dram_tensor("x0n", [RPC, D], bf16, kind="ExternalInput").ap()
    x1n = nc.dram_tensor("x1n", [RPC, D], bf16, kind="ExternalInput").ap()
    x0t = nc.dram_tensor("x0t", [D, RPC], bf16, kind="ExternalInput").ap()
    x1t = nc.dram_tensor("x1t", [D, RPC], bf16, kind="ExternalInput").ap()
    tt = nc.dram_tensor("tt", [D, B], bf16, kind="ExternalInput").ap()
    lab = nc.dram_tensor("lab", [RPC, B], bf16, kind="ExternalInput").ap()
    out = nc.dram_tensor("out", [P, 1], f32, kind="ExternalOutput").ap()

    with tile.TileContext(nc) as tc:
        with (
            tc.tile_pool(name="big", bufs=1) as big,
            tc.tile_pool(name="stream", bufs=2) as stream,
            tc.tile_pool(name="work", bufs=3) as work,
            tc.tile_pool(name="small", bufs=1) as small,
            tc.tile_pool(name="dram", bufs=1, space="DRAM") as dram,
        ):
            # ---- resident loads
            tt_sb = big.tile([P, KT, B], bf16)
            nc.sync.dma_start(tt_sb[:], tt.rearrange("(o p) j -> p o j", p=P))
            lab_sb = big.tile([P, ITILES, B], bf16)
            nc.sync.dma_start(lab_sb[:], lab.rearrange("(o p) j -> p o j", p=P))
            xt_sb = []
            for mi, xt in enumerate((x0t, x1t)):
                t_ = big.tile([P, KT, RPC], bf16, tag=f"xt{mi}")
                nc.sync.dma_start(t_[:], xt.rearrange("(o p) i -> p o i", p=P))
                xt_sb.append(t_)

            ones_bf = small.tile([P, 1], bf16)
            nc.vector.memset(ones_bf[:], 1.0)
            neg1 = small.tile([P, 1], f32)
            nc.vector.memset(neg1[:], -1.0)

            # ---- X row norms from natural layout: fx = 1/max(sqrt(ssq),eps)
            fx, negfx = [], []
            for mi, xn in enumerate((x0n, x1n)):
                ssq = small.tile([P, ITILES], f32, tag=f"xssq{mi}")
                for it in range(ITILES):
                    xn_sb = stream.tile([P, D], bf16, tag="xn")
                    nc.sync.dma_start(
                        xn_sb[:],
                        xn.rearrange("(o p) d -> p o d", p=P)[:, it])
                    sq_scr = stream.tile([P, D], bf16, tag="xsq_scr")
                    nc.scalar.activation(
                        sq_scr[:], xn_sb[:], AF.Square,
                        accum_out=ssq[:, it : it + 1])
                fxm = small.tile([P, ITILES], f32, tag=f"fx{mi}")
                nfxm = small.tile([P, ITILES], f32, tag=f"nfx{mi}")
                nc.scalar.sqrt(fxm[:], ssq[:])
                nc.vector.tensor_scalar(fxm[:], fxm[:], EPS, None, A.max)
                nc.vector.reciprocal(fxm[:], fxm[:])
                nc.vector.tensor_scalar_mul(nfxm[:], fxm[:], -1.0)
                fx.append(fxm)
                negfx.append(nfxm)

            # ---- T col norms (transposed layout): ones-matmul over squares
            bounce = dram.tile([1, B], bf16)
            with tc.tile_pool(name="pps", bufs=1, space="PSUM") as pps:
                tss_ps = [pps.tile([1, 512], f32, tag=f"tss{n}", name=f"tss{n}")
                          for n in range(8)]
                for k in range(KT):
                    tsq = stream.tile([P, B], bf16, tag="tsq")
                    nc.vector.tensor_tensor(tsq[:], tt_sb[:, k], tt_sb[:, k], A.mult)
                    for n in range(8):
                        nc.tensor.matmul(
                            tss_ps[n][:], ones_bf[:], tsq[:, n * 512 : (n + 1) * 512],
                            start=(k == 0), stop=(k == KT - 1))
                for n in range(8):
                    ft_f = small.tile([1, 512], f32, tag="ft_f")
                    nc.scalar.sqrt(ft_f[:], tss_ps[n][:])
                    nc.vector.tensor_scalar(ft_f[:], ft_f[:], EPS, None, A.max)
                    nc.vector.reciprocal(ft_f[:], ft_f[:])
                    ft_b = small.tile([1, 512], bf16, tag="ft_b")
                    nc.vector.tensor_copy(ft_b[:], ft_f[:])
                    nc.sync.dma_start(bounce[:, n * 512 : (n + 1) * 512], ft_b[:])
            # broadcast [1,B] -> [P,B] via DRAM bounce
            fbc = big.tile([P, B], bf16)
            bc_ap = bass.AP(tensor=bounce.tensor, offset=bounce.offset,
                            ap=[[0, P]] + list(bounce.ap))
            nc.sync.dma_start(fbc[:], bc_ap)
            # normalize tt in place
            for k in range(KT):
                nc.vector.tensor_tensor(tt_sb[:, k], tt_sb[:, k], fbc[:], A.mult)

            # ---- sum(labels) per partition
            lsum = small.tile([P, 1], f32)
            nc.vector.tensor_reduce(lsum[:], lab_sb[:], mybir.AxisListType.XY, A.add)

            # ---- main: c = x^T-tile.T @ tt-tile, fused loss passes
            NSLOT = 2 * ITILES * NJ  # 64
            racc = small.tile([P, NSLOT], f32)
            hacc = small.tile([P, NSLOT], f32)
            gacc = small.tile([P, NSLOT], f32)
            with tc.tile_pool(name="cps", bufs=2, space="PSUM") as cps:
                for mi in range(2):
                    for it in range(ITILES):
                        for jh in range(2):
                            c_ps = cps.tile([P, 4 * 512], f32, tag="c")
                            for k in range(KT):
                                for j4 in range(4):
                                    j = jh * 4 + j4
                                    nc.tensor.matmul(
                                        c_ps[:, j4 * 512 : (j4 + 1) * 512],
                                        xt_sb[mi][:, k, it * P : (it + 1) * P],
                                        tt_sb[:, k, j * 512 : (j + 1) * 512],
                                        start=(k == 0), stop=(k == KT - 1))
                            for j4 in range(4):
                                j = jh * 4 + j4
                                slot = ((mi * ITILES + it) * 2 + jh) * 4 + j4
                                cpsj = c_ps[:, j4 * 512 : (j4 + 1) * 512]
                                labj = lab_sb[:, it, j * 512 : (j + 1) * 512]
                                r_t = work.tile([P, 512], bf16, tag="r")
                                nc.scalar.activation(
                                    r_t[:], cpsj, AF.Relu,
                                    bias=neg1[:], scale=fx[mi][:, it : it + 1],
                                    accum_out=racc[:, slot : slot + 1])
                                h_t = work.tile([P, 512], bf16, tag="h")
                                nc.vector.scalar_tensor_tensor(
                                    out=h_t[:], in0=cpsj,
                                    scalar=negfx[mi][:, it : it + 1], in1=labj,
                                    op0=A.mult, op1=A.mult,
                                    accum_out=hacc[:, slot : slot + 1])
                                g_t = work.tile([P, 512], bf16, tag="g")
                                nc.vector.scalar_tensor_tensor(
                                    out=g_t[:], in0=r_t[:], scalar=1.0, in1=labj,
                                    op0=A.mult, op1=A.mult,
                                    accum_out=gacc[:, slot : slot + 1])

            # ---- combine partials: tot = 2*lsum + sum(hacc) + sum(racc) - sum(gacc)
            hred = small.tile([P, 1], f32)
            rred = small.tile([P, 1], f32)
            gred = small.tile([P, 1], f32)
            nc.vector.tensor_reduce(hred[:], hacc[:], mybir.AxisListType.X, A.add)
            nc.vector.tensor_reduce(rred[:], racc[:], mybir.AxisListType.X, A.add)
            nc.vector.tensor_reduce(gred[:], gacc[:], mybir.AxisListType.X, A.add)
            tot = small.tile([P, 1], f32)
            nc.vector.scalar_tensor_tensor(
                out=tot[:], in0=lsum[:], scalar=2.0, in1=hred[:],
                op0=A.mult, op1=A.add)
            nc.vector.tensor_tensor(tot[:], tot[:], rred[:], A.add)
            nc.vector.tensor_tensor(tot[:], tot[:], gred[:], A.subtract)
            nc.sync.dma_start(out, tot[:])

    _split_waits(nc, max_waits=1)
    return nc


def _get_nc():
    if "nc" not in _CACHE:
        _CACHE["nc"] = _build()
    return _CACHE["nc"]


def kernel(fc_feats_0, fc_feats_1, textual_features, labels):
    nc = _get_nc()
    bf = ml_dtypes.bfloat16
    f0 = np.asarray(fc_feats_0, dtype=np.float32)
    f1 = np.asarray(fc_feats_1, dtype=np.float32)
    t = np.asarray(textual_features, dtype=np.float32)
    lb = np.asarray(labels, dtype=np.float32)

    f0b = f0.astype(bf)
    f1b = f1.astype(bf)
    f0tb = np.ascontiguousarray(f0.T.astype(bf))
    f1tb = np.ascontiguousarray(f1.T.astype(bf))
    ttb = np.ascontiguousarray(t.T.astype(bf))
    lbb = lb.astype(bf)

    in_maps = []
    for m in range(NCORES):
        s = slice(m * RPC, (m + 1) * RPC)
        in_maps.append(dict(
            x0n=np.ascontiguousarray(f0b[s]),
            x1n=np.ascontiguousarray(f1b[s]),
            x0t=np.ascontiguousarray(f0tb[:, s]),
            x1t=np.ascontiguousarray(f1tb[:, s]),
            tt=ttb,
            lab=np.ascontiguousarray(lbb[s]),
        ))
    res = bass_utils.run_bass_kernel_spmd(nc, in_maps, list(range(NCORES)))
    total = np.float64(0.0)
    for r in res.results:
        total += np.float64(r["out"].sum(dtype=np.float64))
    return np.float32(total / (B * B))


# revision 11
# speedup vs baseline: 1.2519x; 1.2519x over previous
"""ContrastiveLoss distributed Trainium2 kernel (8 NeuronCores).

Math (reference):
  t = l2norm(textual); c0 = l2norm(f0) @ t.T; c1 = l2norm(f1) @ t.T
  loss = sum(lab*(1-c) + (1-lab)*relu(c-1)) over both c / B^2

Sharding: rows of f0/f1/labels split across 8 cores (512 rows each);
textual replicated. Per core, per similarity matrix:
  sum = sum(lab) - sum(lab*c) + sum(r) - sum(lab*r),   r = relu(c-1)
Per-core partial sums returned as [128,1] per-partition totals; host sums.

Device layout: host ships bf16 transposed operands (layout/dtype marshalling
only; all float math on device):
  x{0,1}n [512,1024]  natural (row sumsq via ACT Square+accum)
  x{0,1}t [1024,512]  transposed (matmul lhsT tiles, unnormalized)
  tt      [1024,4096] transposed textual (matmul rhs, normalized in-place)
  lab     [512,4096]  labels slice
X normalization is folded into the per-partition scale operand of the
ACT/DVE passes over PSUM (c_raw*fx = c), so x*t tiles stay unnormalized.
"""
import sys

if "/opt/trn_rl_repo" not in sys.path:
    sys.path.insert(0, "/opt/trn_rl_repo")

import numpy as np
import ml_dtypes

import concourse.bass as bass
import concourse.mybir as mybir
import concourse.tile as tile
from concourse import bass_utils
import bass_rust

B, D = 4096, 1024
NCORES = 8
RPC = B // NCORES          # rows per core = 512
P = 128
ITILES = RPC // P          # 4
KT = D // P                # 8 contraction tiles
NJ = B // 512              # 8 j-chunks of 512
bf16 = mybir.dt.bfloat16
f32 = mybir.dt.float32
EPS = 1e-8

_CACHE = {}


def _split_waits(nc, max_waits=1):
    """This walrus build rejects >1 semaphore wait per instruction; hoist
    extras onto same-engine NOPs placed immediately before."""
    SI = bass_rust.SyncInfo
    n = 0
    for bb in nc.main_func.blocks:
        new_insts, changed = [], False
        for inst in bb.instructions:
            si = inst.sync_info
            if si is None:
                new_insts.append(inst)
                continue
            waits = list(si.on_wait)
            if len(waits) > max_waits:
                extra, keep = waits[:-max_waits], waits[-max_waits:]
                for j in range(0, len(extra), max_waits):
                    nop = mybir.InstNoOp(name=f"{inst.name}-ws{j}", ins=[], outs=[])
                    nop.engine = inst.engine
                    nop.sync_info = SI(on_wait=extra[j : j + max_waits], on_update=[])
                    nc.register_instruction(nop, overwrite=True)
                    new_insts.append(nop)
                    n += 1
                inst.sync_info = SI(on_wait=keep, on_update=list(si.on_update))
                changed = True
            new_insts.append(inst)
        if changed:
            bb.instructions = new_insts
    return n


def _build():
    nc = bass.Bass("TRN2", target_bir_lowering=False, debug=False,
                   num_devices=NCORES)
    A = mybir.AluOpType
    AF = mybir.ActivationFunctionType

    x0n = nc.dram_tensor("x0n", [RPC, D], bf16, kind="ExternalInput").ap()
    x1n = nc.dram_tensor("x1n", [RPC, D], bf16, kind="ExternalInput").ap()
    x0t = nc.dram_tensor("x0t", [D, RPC], bf16, kind="ExternalInput").ap()
    x1t = nc.dram_tensor("x1t", [D, RPC], bf16, kind="ExternalInput").ap()
    tt = nc.dram_tensor("tt", [D, B], bf16, kind="ExternalInput").ap()
    lab = nc.dram_tensor("lab", [RPC, B], bf16, kind="ExternalInput").ap()
    out = nc.dram_tensor("out", [P, 1], f32, kind="ExternalOutput").ap()

    with tile.TileContext(nc) as tc:
        with (
            tc.tile_pool(name="big", bufs=1) as big,
            tc.tile_pool(name="stream", bufs=2) as stream,
            tc.tile_pool(name="work", bufs=3) as work,
            tc.tile_pool(name="small", bufs=1) as small,
            tc.tile_pool(name="dram", bufs=1, space="DRAM") as dram,
        ):
            # ---- resident loads (tt split per k-tile so normalization and
            # main matmuls pipeline instead of serializing on one big tile)
            tt_re = tt.rearrange("(o p) j -> p o j", p=P)
            tt_k = []
            for k in range(KT):
                tk = big.tile([P, B], bf16, tag=f"ttk{k}", name=f"ttk{k}")
                nc.sync.dma_start(tk[:], tt_re[:, k])
                tt_k.append(tk)
            lab_sb = big.tile([P, ITILES, B], bf16)
            nc.sync.dma_start(lab_sb[:], lab.rearrange("(o p) j -> p o j", p=P))
            xt_sb = []
            for mi, xt in enumerate((x0t, x1t)):
                t_ = big.tile([P, KT, RPC], bf16, tag=f"xt{mi}")
                nc.sync.dma_start(t_[:], xt.rearrange("(o p) i -> p o i", p=P))
                xt_sb.append(t_)

            ones_bf = small.tile([P, 1], bf16)
            nc.vector.memset(ones_bf[:], 1.0)
            neg1 = small.tile([P, 1], f32)
            nc.vector.memset(neg1[:], -1.0)

            # ---- X row norms from natural layout: fx = 1/max(sqrt(ssq),eps)
            fx, negfx = [], []
            for mi, xn in enumerate((x0n, x1n)):
                ssq = small.tile([P, ITILES], f32, tag=f"xssq{mi}")
                for it in range(ITILES):
                    xn_sb = stream.tile([P, D], bf16, tag="xn")
                    nc.sync.dma_start(
                        xn_sb[:],
                        xn.rearrange("(o p) d -> p o d", p=P)[:, it])
                    sq_scr = stream.tile([P, D], bf16, tag="xsq_scr")
                    nc.scalar.activation(
                        sq_scr[:], xn_sb[:], AF.Square,
                        accum_out=ssq[:, it : it + 1])
                fxm = small.tile([P, ITILES], f32, tag=f"fx{mi}")
                nfxm = small.tile([P, ITILES], f32, tag=f"nfx{mi}")
                nc.scalar.sqrt(fxm[:], ssq[:])
                nc.vector.tensor_scalar(fxm[:], fxm[:], EPS, None, A.max)
                nc.vector.reciprocal(fxm[:], fxm[:])
                nc.vector.tensor_scalar_mul(nfxm[:], fxm[:], -1.0)
                fx.append(fxm)
                negfx.append(nfxm)

            # ---- T col norms (transposed layout): ones-matmul over squares
            bounce = dram.tile([1, B], bf16)
            with tc.tile_pool(name="pps", bufs=1, space="PSUM") as pps:
                tss_ps = [pps.tile([1, 512], f32, tag=f"tss{n}", name=f"tss{n}")
                          for n in range(8)]
                for k in range(KT):
                    tsq = stream.tile([P, B], bf16, tag="tsq")
                    nc.vector.tensor_tensor(tsq[:], tt_k[k][:], tt_k[k][:], A.mult)
                    for n in range(8):
                        nc.tensor.matmul(
                            tss_ps[n][:], ones_bf[:], tsq[:, n * 512 : (n + 1) * 512],
                            start=(k == 0), stop=(k == KT - 1))
                for n in range(8):
                    ft_f = small.tile([1, 512], f32, tag="ft_f")
                    nc.scalar.sqrt(ft_f[:], tss_ps[n][:])
                    nc.vector.tensor_scalar(ft_f[:], ft_f[:], EPS, None, A.max)
                    nc.vector.reciprocal(ft_f[:], ft_f[:])
                    ft_b = small.tile([1, 512], bf16, tag="ft_b")
                    nc.vector.tensor_copy(ft_b[:], ft_f[:])
                    nc.sync.dma_start(bounce[:, n * 512 : (n + 1) * 512], ft_b[:])
            # broadcast [1,B] -> [P,B] via DRAM bounce
            fbc = big.tile([P, B], bf16)
            bc_ap = bass.AP(tensor=bounce.tensor, offset=bounce.offset,
                            ap=[[0, P]] + list(bounce.ap))
            nc.sync.dma_start(fbc[:], bc_ap)
            # normalize tt in place, per k-tile
            for k in range(KT):
                nc.vector.tensor_tensor(tt_k[k][:], tt_k[k][:], fbc[:], A.mult)

            # ---- sum(labels) per partition
            lsum = small.tile([P, 1], f32)
            nc.vector.tensor_reduce(lsum[:], lab_sb[:], mybir.AxisListType.XY, A.add)

            # ---- main: c = x^T-tile.T @ tt-tile, fused loss passes
            NSLOT = 2 * ITILES * NJ  # 64
            racc = small.tile([P, NSLOT], f32)
            hacc = small.tile([P, NSLOT], f32)
            gacc = small.tile([P, NSLOT], f32)
            with tc.tile_pool(name="cps", bufs=2, space="PSUM") as cps:
                for mi in range(2):
                    for it in range(ITILES):
                        for jh in range(2):
                            c_ps = cps.tile([P, 4 * 512], f32, tag="c")
                            for k in range(KT):
                                for j4 in range(4):
                                    j = jh * 4 + j4
                                    nc.tensor.matmul(
                                        c_ps[:, j4 * 512 : (j4 + 1) * 512],
                                        xt_sb[mi][:, k, it * P : (it + 1) * P],
                                        tt_k[k][:, j * 512 : (j + 1) * 512],
                                        start=(k == 0), stop=(k == KT - 1))
                            for j4 in range(4):
                                j = jh * 4 + j4
                                slot = ((mi * ITILES + it) * 2 + jh) * 4 + j4
                                cpsj = c_ps[:, j4 * 512 : (j4 + 1) * 512]
                                labj = lab_sb[:, it, j * 512 : (j + 1) * 512]
                                r_t = work.tile([P, 512], bf16, tag="r")
                                nc.scalar.activation(
                                    r_t[:], cpsj, AF.Relu,
                                    bias=neg1[:], scale=fx[mi][:, it : it + 1],
                                    accum_out=racc[:, slot : slot + 1])
                                h_t = work.tile([P, 512], bf16, tag="h")
                                nc.vector.scalar_tensor_tensor(
                                    out=h_t[:], in0=cpsj,
                                    scalar=negfx[mi][:, it : it + 1], in1=labj,
                                    op0=A.mult, op1=A.mult,
                                    accum_out=hacc[:, slot : slot + 1])
                                g_t = work.tile([P, 512], bf16, tag="g")
                                nc.vector.scalar_tensor_tensor(
                                    out=g_t[:], in0=r_t[:], scalar=1.0, in1=labj,
                                    op0=A.mult, op1=A.mult,
                                    accum_out=gacc[:, slot : slot + 1])

            # ---- combine partials: tot = 2*lsum + sum(hacc) + sum(racc) - sum(gacc)
            hred = small.tile([P, 1], f32)
            rred = small.tile([P, 1], f32)
            gred = small.tile([P, 1], f32)
            nc.vector.tensor_reduce(hred[:], hacc[:], mybir.AxisListType.X, A.add)
            nc.vector.tensor_reduce(rred[:], racc[:], mybir.AxisListType.X, A.add)
            nc.vector.tensor_reduce(gred[:], gacc[:], mybir.AxisListType.X, A.add)
            tot = small.tile([P, 1], f32)
            nc.vector.scalar_tensor_tensor(
                out=tot[:], in0=lsum[:], scalar=2.0, in1=hred[:],
                op0=A.mult, op1=A.add)
            nc.vector.tensor_tensor(tot[:], tot[:], rred[:], A.add)
            nc.vector.tensor_tensor(tot[:], tot[:], gred[:], A.subtract)
            nc.sync.dma_start(out, tot[:])

    _split_waits(nc, max_waits=1)
    return nc


def _get_nc():
    if "nc" not in _CACHE:
        _CACHE["nc"] = _build()
    return _CACHE["nc"]


def kernel(fc_feats_0, fc_feats_1, textual_features, labels):
    nc = _get_nc()
    bf = ml_dtypes.bfloat16
    f0 = np.asarray(fc_feats_0, dtype=np.float32)
    f1 = np.asarray(fc_feats_1, dtype=np.float32)
    t = np.asarray(textual_features, dtype=np.float32)
    lb = np.asarray(labels, dtype=np.float32)

    f0b = f0.astype(bf)
    f1b = f1.astype(bf)
    f0tb = np.ascontiguousarray(f0.T.astype(bf))
    f1tb = np.ascontiguousarray(f1.T.astype(bf))
    ttb = np.ascontiguousarray(t.T.astype(bf))
    lbb = lb.astype(bf)

    in_maps = []
    for m in range(NCORES):
        s = slice(m * RPC, (m + 1) * RPC)
        in_maps.append(dict(
            x0n=np.ascontiguousarray(f0b[s]),
            x1n=np.ascontiguousarray(f1b[s]),
            x0t=np.ascontiguousarray(f0tb[:, s]),
            x1t=np.ascontiguousarray(f1tb[:, s]),
            tt=ttb,
            lab=np.ascontiguousarray(lbb[s]),
        ))
    res = bass_utils.run_bass_kernel_spmd(nc, in_maps, list(range(NCORES)))
    total = np.float64(0.0)
    for r in res.results:
        total += np.float64(r["out"].sum(dtype=np.float64))
    return np.float32(total / (B * B))
